# revision 1
# baseline (speedup 1.0000x reference)
"""Trainium2 Bass kernel for biased multi-head attention (nn_Attention_42949673623).

Computation (reference):
    t = x @ W_proj.T                      # (B,L,768) fused QKV
    q,k,v per head (H=8, hw=32), q *= hw**-0.5
    a = softmax(q @ k.T + bias.transpose(0,3,1,2), axis=-1)
    y = a @ v                             # (B,H,L,hw) -> (B,L,256)
    out = y @ W_o.T + b_o

Sharding: B(2) x H(8) = 16 (batch, head) pairs over 8 cores, 2 heads per core.
Each core computes its two heads' attention and a partial output projection
(64 of the 256 contraction channels); the host sums the 4 partials per batch.

Per-core device layout (transposed so softmax k-dim lands on partitions and
the per-head bias slab is DMA-contiguous):
    xT     (256, 2048)  x[b].T
    wqkvT  (256, 192)   W_proj rows for 2 heads, transposed, q-scaled
    woT    (64, 256)    W_o columns for this core's 64 channels, transposed
    biasT  (2, 2048, 2048)  bias[b,:,:,h].T per head
    outT   (256, 2048)  partial (y @ W_o.T).T for batch b
"""

import re

import numpy as np

B, L, E, H, HW = 2, 2048, 256, 8, 32
NCORES = 8
HEADS_PER_CORE = 2
P = 128
NTILES = L // P  # 16 key tiles
NCH = 4          # 512-wide free chunks per 2048
FREE = 512

_PATCHED = [False]
_CACHE = {}


def _patch_tile_drain():
    """The walrus codegen in this toolchain caps sync-waits per instruction
    (1 for matmul, 2 otherwise). TileContext's tail drain waits on every live
    semaphore at once; replace it with explicit single-wait instructions."""
    if _PATCHED[0]:
        return
    import concourse.tile as tile_mod

    def _drain_and_barrier(self, tick_clock, wait_clock):
        nc = self.nc
        ticks = [int(v) for v in re.findall(r"\d+", repr(tick_clock.global_clock))]
        for proc_idx, sem in sorted(self.sems.allocated().items()):
            if proc_idx < len(ticks) and ticks[proc_idx] > 0:
                mult = 16 if sem.name.startswith("DMA") else 1
                nc.sync.wait_ge(sem, ticks[proc_idx] * mult)
        nc.sync.drain()
        nc.all_engine_barrier()
        popped = nc._tile_sem_poison_stack.pop()
        assert popped is self._sem_poison
        nc.clear_and_free_semaphores(list(self.sems.allocated().values()))
        nc.all_engine_barrier()

    tile_mod.TileContext._drain_and_barrier = _drain_and_barrier
    _PATCHED[0] = True


def _split_excess_waits(nc):
    """Move excess per-instruction sem waits onto preceding same-engine nops."""
    import bass_rust
    import concourse.mybir as mybir

    counter = [0]
    for f in nc.m.functions:
        for blk in f.blocks:
            out, changed = [], False
            for inst in blk.instructions:
                si = inst.sync_info
                if si is not None and si.on_wait and len(si.on_wait) > 1:
                    waits = list(si.on_wait)
                    extra, keep = waits[:-1], waits[-1:]
                    for w in extra:
                        counter[0] += 1
                        nop = mybir.InstNoOp(
                            name=f"I-wsplit{counter[0]}", ins=[], outs=[]
                        )
                        nop.engine = inst.engine
                        nop.sync_info = bass_rust.SyncInfo(
                            on_wait=[w], on_update=[]
                        )
                        out.append(nop)
                    inst.sync_info = bass_rust.SyncInfo(
                        on_wait=keep, on_update=list(si.on_update)
                    )
                    changed = True
                out.append(inst)
            if changed:
                blk.instructions = out


INJECT_SET = frozenset(
    [(0, 0), (0, 1), (1, 0), (1, 1), (1, 2), (1, 15)]
)
QKV1_AT = 13


def build(reps: int = 1, split_waits: bool = True, inject_set=INJECT_SET):
    """Build the SPMD Bass program (identical on all 8 cores)."""
    import concourse.bass as bass
    import concourse.mybir as mybir
    from concourse.tile import TileContext

    _patch_tile_drain()
    F32 = mybir.dt.float32
    F32R = mybir.dt.float32r
    EXP = mybir.ActivationFunctionType.Exp
    MULT = mybir.AluOpType.mult
    ADD = mybir.AluOpType.add

    nc = bass.Bass()
    xT = nc.declare_dram_parameter("xT", (E, L), F32, isOutput=False)
    wqkvT = nc.declare_dram_parameter("wqkvT", (E, 96 * HEADS_PER_CORE), F32, isOutput=False)
    woT = nc.declare_dram_parameter("woT", (HW * HEADS_PER_CORE, E), F32, isOutput=False)
    biasT = nc.declare_dram_parameter("biasT", (HEADS_PER_CORE, L, L), F32, isOutput=False)
    outT = nc.declare_dram_parameter("outT", (E, L), F32, isOutput=True)

    with TileContext(nc) as tc:
        with (
            tc.tile_pool(name="sb", bufs=1) as sb,
            tc.tile_pool(name="sbias", bufs=5) as sbias,
            tc.tile_pool(name="sbias_r", bufs=3) as sbias_r,
            tc.tile_pool(name="swork", bufs=6) as swork,
            tc.tile_pool(name="ps", bufs=2, space="PSUM") as ps,
            tc.tile_pool(name="psacc", bufs=2, space="PSUM") as psacc,
        ):
            for _ in range(reps):
                # ---- load + round inputs -------------------------------
                wr = []
                for e in range(2):
                    w_r = sb.tile([P, 96 * HEADS_PER_CORE], F32R, tag=f"wtr{e}")
                    nc.gpsimd.dma_start(out=w_r[:], in_=wqkvT[e * P : (e + 1) * P, :])
                    wr.append(w_r)
                wor = sb.tile([HW * HEADS_PER_CORE, E], F32R, tag="wor")
                nc.gpsimd.dma_start(out=wor[:], in_=woT[:])
                xtr = []
                for e in range(2):
                    xr = sb.tile([P, L], F32R, tag=f"xtr{e}")
                    xtr.append(xr)
                for hf in range(2):
                    for e in range(2):
                        nc.gpsimd.dma_start(
                            out=xtr[e][:, hf * (L // 2) : (hf + 1) * (L // 2)],
                            in_=xT[e * P : (e + 1) * P, hf * (L // 2) : (hf + 1) * (L // 2)],
                        )
                # f32r identity for PE bias injection
                identf = sb.tile([P, P], F32, tag="identf")
                from concourse.masks import make_identity
                make_identity(nc, identf[:])
                ident = sb.tile([P, P], F32R, tag="ident")
                nc.scalar.copy(out=ident[:], in_=identf[:])

                # prefetch first bias tiles right after the input loads
                bias_tiles = {}

                # persistent reciprocal staging tiles (zeroed once)
                rs0 = sb.tile([HW, L // 2], F32, tag="rs0")
                rs1 = sb.tile([HW, L // 2], F32, tag="rs1")
                nc.vector.memset(rs0[:], 0.0)
                nc.vector.memset(rs1[:], 0.0)
                rsT = [rs0, rs1]

                # ---- per-head pipeline: QKV then attention k-loop ------
                qT, kT, vAll = {}, {}, {}

                def emit_qkv(h):
                    copy_eng = nc.vector.tensor_copy if h == 0 else None
                    qk = []
                    for which in range(2):  # 0 -> q, 1 -> k
                        t = sb.tile([HW, L], F32R, tag=f"qk{h}{which}", name=f"qk{h}{which}")
                        c0 = h * 96 + which * HW
                        for hf in range(2):
                            q0 = hf * (L // 2)
                            pq = ps.tile([HW, L // 2], F32, tag="st", name=f"pq{h}{which}{hf}")
                            for n in range(2):
                                for e in range(2):
                                    nc.tensor.matmul(
                                        pq[:, n * FREE : (n + 1) * FREE],
                                        wr[e][:, c0 : c0 + HW],
                                        xtr[e][:, q0 + n * FREE : q0 + (n + 1) * FREE],
                                        start=(e == 0),
                                        stop=(e == 1),
                                    )
                            if h == 0 or hf == 0:
                                nc.vector.tensor_copy(out=t[:, q0 : q0 + L // 2], in_=pq[:])
                            else:
                                nc.scalar.copy(out=t[:, q0 : q0 + L // 2], in_=pq[:])
                        qk.append(t)
                    qT[h] = qk[0]
                    kT[h] = qk[1]

                    # V: (l, c) tiles packed with a ones column (33rd) per tile
                    va = sb.tile([P, NTILES * (HW + 1)], F32R, tag=f"vall{h}", name=f"vall{h}")
                    # ones columns (33rd of each V tile): out = 0*in + 1, f32r-rounded
                    nc.scalar.activation(
                        out=va[:].rearrange("p (t c) -> p t c", c=HW + 1)[:, :, HW : HW + 1],
                        in_=xtr[0][:, 0:NTILES, None],
                        func=mybir.ActivationFunctionType.Identity,
                        scale=0.0,
                        bias=1.0,
                    )
                    c0 = h * 96 + 2 * HW
                    for half in range(2):  # 8 l-tiles per PSUM bank
                        pv = ps.tile([P, 8 * HW], F32, tag="st", name=f"pv{h}{half}")
                        for i in range(8):
                            lt = half * 8 + i
                            for e in range(2):
                                nc.tensor.matmul(
                                    pv[:, i * HW : (i + 1) * HW],
                                    xtr[e][:, lt * P : (lt + 1) * P],
                                    wr[e][:, c0 : c0 + HW],
                                    start=(e == 0),
                                    stop=(e == 1),
                                )
                        # strided eviction into the 33-stride layout
                        out_ap = va[:, half * 8 * (HW + 1) :].rearrange(
                            "p (t c) -> p t c", c=HW + 1
                        )[:, 0:8, 0:HW]
                        in_ap = pv[:].rearrange("p (t c) -> p t c", c=HW)[:, 0:8, :]
                        if h == 0:
                            nc.vector.tensor_copy(out=out_ap, in_=in_ap)
                        else:
                            nc.scalar.activation(
                                out=out_ap, in_=in_ap,
                                func=mybir.ActivationFunctionType.Copy,
                            )
                    vAll[h] = va

                ytn = sb.tile([HEADS_PER_CORE * HW, L], F32R, tag="ytn")
                for h in range(HEADS_PER_CORE):
                    if h == 0:
                        emit_qkv(h)
                    py0 = psacc.tile([HW + 1, L // 2], F32, tag="acc")
                    py1 = psacc.tile([HW + 1, L // 2], F32, tag="acc")
                    py = [py0, py1]
                    for kt in range(NTILES):
                        inject = (h, kt) in inject_set
                        if inject:
                            # f32r bias via SWDGE cast; PE identity-matmul adds
                            # it into PSUM, then QK^T accumulates on top.
                            btr = sbias_r.tile([P, L], F32R, tag="biasr")
                            nc.gpsimd.dma_start(
                                out=btr[:], in_=biasT[h, kt * P : (kt + 1) * P, :]
                            )
                        else:
                            bt = sbias.tile([P, L], F32, tag="bias")
                            nc.sync.dma_start(
                                out=bt[:], in_=biasT[h, kt * P : (kt + 1) * P, :]
                            )
                        # process in 1024-wide q-halves so the S^T PSUM slab
                        # (2 banks each, bufs=2) double-buffers: PE S-matmuls
                        # of one half overlap the DVE add of the other.
                        for hf in range(2):
                            q0 = hf * (L // 2)
                            pst = ps.tile([P, L // 2], F32, tag="st")
                            for n in range(2):
                                if inject:
                                    nc.tensor.matmul(
                                        pst[:, n * FREE : (n + 1) * FREE],
                                        ident[:],
                                        btr[:, q0 + n * FREE : q0 + (n + 1) * FREE],
                                        start=True,
                                        stop=False,
                                    )
                                nc.tensor.matmul(
                                    pst[:, n * FREE : (n + 1) * FREE],
                                    kT[h][:, kt * P : (kt + 1) * P],
                                    qT[h][:, q0 + n * FREE : q0 + (n + 1) * FREE],
                                    start=not inject,
                                    stop=True,
                                )
                            p_sb = swork.tile([P, L // 2], F32R, tag="p_sb")
                            if inject:
                                nc.scalar.activation(out=p_sb[:], in_=pst[:], func=EXP)
                            else:
                                nc.vector.tensor_tensor(
                                    p_sb[:], pst[:], bt[:, q0 : q0 + L // 2], ADD
                                )
                                nc.scalar.activation(out=p_sb[:], in_=p_sb[:], func=EXP)
                            for n in range(2):
                                nc.tensor.matmul(
                                    py[hf][:, n * FREE : (n + 1) * FREE],
                                    vAll[h][:, kt * (HW + 1) : (kt + 1) * (HW + 1)],
                                    p_sb[:, n * FREE : (n + 1) * FREE],
                                    start=(kt == 0),
                                    stop=(kt == NTILES - 1),
                                )
                        if h == 0 and kt == QKV1_AT:
                            # emit head-1 QKV here: it fills PE/copy slack in
                            # head-0's steady stream, so the head transition
                            # only costs the normalize.
                            emit_qkv(1)
                    # normalize per half: y^T[c,q] = ytilde^T[c,q] / s[q]
                    for hf in range(2):
                        q0 = hf * (L // 2)
                        rs = rsT[hf]
                        nc.vector.reciprocal(
                            out=rs[0:1, :], in_=py[hf][HW : HW + 1, :]
                        )
                        rb = sb.tile([HW, L // 2], F32, tag="rb")
                        nc.vector.stream_shuffle(rb[:], rs[:], [0] * 32)
                        nc.vector.tensor_tensor(
                            ytn[h * HW : (h + 1) * HW, q0 : q0 + L // 2],
                            py[hf][0:HW, :],
                            rb[:],
                            MULT,
                        )

                # ---- phase 3: partial output projection ----------------
                for hf in range(2):
                    q0 = hf * (L // 2)
                    for fc in range(2):
                        po = ps.tile([P, L // 2], F32, tag="st")
                        for n in range(2):
                            nc.tensor.matmul(
                                po[:, n * FREE : (n + 1) * FREE],
                                wor[:, fc * P : (fc + 1) * P],
                                ytn[:, q0 + n * FREE : q0 + (n + 1) * FREE],
                                start=True,
                                stop=True,
                            )
                        o_sb = swork.tile([P, L // 2], F32, tag="o_sb")
                        if fc == 0:
                            nc.scalar.copy(out=o_sb[:], in_=po[:])
                        else:
                            nc.vector.tensor_copy(out=o_sb[:], in_=po[:])
                        eng = nc.sync if fc == 0 else nc.gpsimd
                        eng.dma_start(
                            out=outT[fc * P : (fc + 1) * P, q0 : q0 + L // 2],
                            in_=o_sb[:],
                        )

    if split_waits:
        _split_excess_waits(nc)
    return nc


def make_in_maps(x, bias, W_proj, W_o):
    """Shard full inputs into the 8 per-core input dicts."""
    x = np.asarray(x, dtype=np.float32)
    bias = np.asarray(bias, dtype=np.float32)
    W_proj = np.asarray(W_proj, dtype=np.float32)
    W_o = np.asarray(W_o, dtype=np.float32)

    scale = np.float32(HW**-0.5)
    in_maps = []
    for core in range(NCORES):
        b = core // 4
        h0 = HEADS_PER_CORE * (core % 4)
        xT = np.ascontiguousarray(x[b].T)
        w = np.array(W_proj[h0 * 96 : (h0 + HEADS_PER_CORE) * 96, :])
        for j in range(HEADS_PER_CORE):
            w[j * 96 : j * 96 + HW] *= scale  # fold q scaling
        wqkvT = np.ascontiguousarray(w.T)
        woT = np.ascontiguousarray(W_o[:, h0 * HW : (h0 + HEADS_PER_CORE) * HW].T)
        biasT = np.ascontiguousarray(
            bias[b].transpose(2, 1, 0)[h0 : h0 + HEADS_PER_CORE]
        )
        in_maps.append({"xT": xT, "wqkvT": wqkvT, "woT": woT, "biasT": biasT})
    return in_maps


def assemble(results, b_o):
    b_o = np.asarray(b_o, dtype=np.float32)
    out = np.zeros((B, L, E), dtype=np.float32)
    for core in range(NCORES):
        b = core // 4
        out[b] += results[core]["outT"].T
    out += b_o
    return out


def run(nc, in_maps):
    from concourse.bass_utils import run_bass_kernel_spmd

    return run_bass_kernel_spmd(nc, in_maps, list(range(NCORES))).results


def kernel(x, bias, W_proj, W_o, b_o):
    key = "nc1"
    if key not in _CACHE:
        _CACHE[key] = build(reps=1)
    nc = _CACHE[key]
    in_maps = make_in_maps(x, bias, W_proj, W_o)
    results = run(nc, in_maps)
    return assemble(results, b_o)

